# revision 6
# baseline (speedup 1.0000x reference)
"""Multi-head 3D attention (8 heads, C=512, N=16^3=4096) on 8 Trainium2 cores.

Sharding: one head per NeuronCore (head-parallel). Each core receives the
full token activations plus its head's slice of the qkv/out projection
weights, computes its head's attention and its partial contribution to the
output projection; the host sums the 8 partial outputs.

Per-core algorithm (all matmuls in f32r = fp22, fp32 accumulate):
  xT   = x.reshape(C, N)                      # [512, 4096], channel-major
  qT   = Wq @ xT, kT = Wk @ xT                # [64, 4096]  (dh on partitions)
  v    = xT.T @ Wv.T                          # [4096, 64]  (keys on partitions)
  S^T  = kT.T-tiles @ qT                      # [128 keys, Nq] PSUM tiles
  P^T  = exp(8 * S^T)                         # ACT, scale fused, no max-sub
  o_aug= [v, 1].T @ P^T                       # [65, Nq]; row 64 = softmax denom
  outp = (w_out_h @ o_aug[:64]) * (1/denom)   # [512, 4096] partial, normalized
The exp needs no max subtraction: logits are < ~80 in magnitude and the HW
exp is accurate over the full fp32 exponent range (verified ~1e-5 rel err).
"""

import sys

for _p in ("/opt/trn_rl_repo",):
    if _p not in sys.path:
        sys.path.insert(0, _p)

import numpy as np

C = 512          # channels
N = 4096         # tokens (16*16*16)
HEADS = 8
DH = C // HEADS  # 64
SCALE = float(DH) ** 0.5  # 8.0 (reference multiplies by sqrt(dh))
NCORES = 8

KT = 128                 # key-tile size (S^T partition dim)
NKT = N // KT            # 32
QG = 2048                # queries per o-psum accumulation group
NQG = N // QG            # 2
SW = 1024                # S-tile width (queries per exp call)
NSW = QG // SW           # 2
MV = 512                 # max fp32 moving free dim per matmul

_compiled = None


def _build():
    import concourse.tile as tile
    from concourse import bacc, mybir

    F32R = mybir.dt.float32r
    F32 = mybir.dt.float32
    EXP = mybir.ActivationFunctionType.Exp

    nc = bacc.Bacc("TRN2", num_devices=NCORES)
    xT_d = nc.dram_tensor("xT", [C, N], F32R, kind="ExternalInput")
    # columns 0:64 = Wq^T, 64:128 = Wk^T, 128:192 = Wv^T (this head's rows)
    wqkvT_d = nc.dram_tensor("wqkvT", [C, 3 * DH], F32R, kind="ExternalInput")
    # w_out[:, head_cols].T  -> [64, 512]
    w_outT_d = nc.dram_tensor("w_outT", [DH, C], F32R, kind="ExternalInput")
    outp_d = nc.dram_tensor("outp", [C, N], F32, kind="ExternalOutput")

    NCT = C // 128  # 4 channel tiles

    with tile.TileContext(nc) as tc:
        with tc.tile_pool(name="const", bufs=1) as const:
            # ---- persistent SBUF tensors ----
            xt = [const.tile([128, N], F32R, tag=f"x{i}", name=f"x{i}") for i in range(NCT)]
            wqkv = [const.tile([128, 3 * DH], F32R, tag=f"w{i}", name=f"w{i}") for i in range(NCT)]
            woutT = const.tile([DH, C], F32R, tag="wo")
            qT = const.tile([DH, N], F32R, tag="qT")
            kT = const.tile([DH, N], F32R, tag="kT")
            vaug = const.tile([128, NKT, DH + 1], F32R, tag="vaug")
            o_sb = const.tile([DH, N], F32R, tag="o")        # unnormalized o^T
            recip = const.tile([1, N], F32, tag="recip")     # 1/denominator
            recipb = const.tile([128, N], F32, tag="recipb")  # broadcast

            for i in range(NCT):
                nc.sync.dma_start(out=xt[i], in_=xT_d.ap()[i * 128:(i + 1) * 128, :])
                nc.sync.dma_start(out=wqkv[i], in_=wqkvT_d.ap()[i * 128:(i + 1) * 128, :])
            nc.sync.dma_start(out=woutT, in_=w_outT_d.ap())
            ones = const.tile([128, 1], F32, tag="ones")
            nc.vector.memset(ones, 1.0)

            # ---- phase 1: projections ----
            with tc.tile_pool(name="ph1", bufs=2, space="PSUM") as ph1:
                # qT / kT: [64, 4096] = Wq/Wk @ xT, by token chunks of 512
                for ch in range(N // MV):
                    sl = slice(ch * MV, (ch + 1) * MV)
                    psq = ph1.tile([DH, MV], F32, tag="psq")
                    psk = ph1.tile([DH, MV], F32, tag="psk")
                    for ct in range(NCT):
                        nc.tensor.matmul(psq, lhsT=wqkv[ct][:, 0:DH],
                                         rhs=xt[ct][:, sl],
                                         start=(ct == 0), stop=(ct == NCT - 1))
                    for ct in range(NCT):
                        nc.tensor.matmul(psk, lhsT=wqkv[ct][:, DH:2 * DH],
                                         rhs=xt[ct][:, sl],
                                         start=(ct == 0), stop=(ct == NCT - 1))
                    nc.vector.tensor_copy(out=qT[:, sl], in_=psq)
                    nc.vector.tensor_copy(out=kT[:, sl], in_=psk)
                # v: [4096, 64] keys-on-partitions, one [128, 64] tile per key tile
                for kt_i in range(NKT):
                    psv = ph1.tile([128, DH], F32, tag="psv")
                    for ct in range(NCT):
                        nc.tensor.matmul(psv,
                                         lhsT=xt[ct][:, kt_i * KT:(kt_i + 1) * KT],
                                         rhs=wqkv[ct][:, 2 * DH:3 * DH],
                                         start=(ct == 0), stop=(ct == NCT - 1))
                    nc.vector.tensor_copy(out=vaug[:, kt_i, 0:DH], in_=psv)
                    nc.vector.tensor_copy(out=vaug[:, kt_i, DH:DH + 1], in_=ones)

            # ---- phase 2: attention ----
            with tc.tile_pool(name="s_ps", bufs=2, space="PSUM") as s_ps, \
                 tc.tile_pool(name="o_ps", bufs=1, space="PSUM") as o_ps, \
                 tc.tile_pool(name="p_sb", bufs=3) as p_sb:
                for qg in range(NQG):
                    q0 = qg * QG
                    ops = o_ps.tile([DH + 1, QG], F32, tag="ops")
                    for kt_i in range(NKT):
                        for sw in range(NSW):
                            s0 = sw * SW
                            sps = s_ps.tile([128, SW], F32, tag="s")
                            for mv in range(SW // MV):
                                nc.tensor.matmul(
                                    sps[:, mv * MV:(mv + 1) * MV],
                                    lhsT=kT[:, kt_i * KT:(kt_i + 1) * KT],
                                    rhs=qT[:, q0 + s0 + mv * MV: q0 + s0 + (mv + 1) * MV],
                                    start=True, stop=True)
                            pt = p_sb.tile([128, SW], F32R, tag="p")
                            nc.scalar.activation(out=pt, in_=sps, func=EXP,
                                                 scale=SCALE)
                            for mv in range(SW // MV):
                                nc.tensor.matmul(
                                    ops[:, s0 + mv * MV: s0 + (mv + 1) * MV],
                                    lhsT=vaug[:, kt_i, :],
                                    rhs=pt[:, mv * MV:(mv + 1) * MV],
                                    start=(kt_i == 0), stop=(kt_i == NKT - 1))
                    # flush: unnormalized o^T and reciprocal of the denominator
                    gsl = slice(q0, q0 + QG)
                    nc.vector.tensor_copy(out=o_sb[:, gsl], in_=ops[0:DH, :])
                    nc.vector.reciprocal(out=recip[:, gsl], in_=ops[DH:DH + 1, :])
                    nc.gpsimd.partition_broadcast(recipb[:, gsl], recip[:, gsl])

            # ---- phase 3: output projection (normalization fused in copy) ----
            with tc.tile_pool(name="out_ps", bufs=2, space="PSUM") as out_ps, \
                 tc.tile_pool(name="out_sb", bufs=3) as out_sb:
                for ct in range(NCT):
                    for ch in range(N // MV):
                        sl = slice(ch * MV, (ch + 1) * MV)
                        pso = out_ps.tile([128, MV], F32, tag="pso")
                        nc.tensor.matmul(pso,
                                         lhsT=woutT[:, ct * 128:(ct + 1) * 128],
                                         rhs=o_sb[:, sl], start=True, stop=True)
                        ot = out_sb.tile([128, MV], F32, tag="ot")
                        nc.vector.tensor_mul(ot, pso, recipb[:, sl])
                        nc.sync.dma_start(
                            out=outp_d.ap()[ct * 128:(ct + 1) * 128, sl], in_=ot)

    nc.compile()
    return nc


def _get_compiled():
    global _compiled
    if _compiled is None:
        _compiled = _build()
    return _compiled


def kernel(x, w_qkv, w_out):
    from concourse.bass_utils import run_bass_kernel_spmd

    x = np.ascontiguousarray(np.asarray(x), dtype=np.float32)
    w_qkv = np.ascontiguousarray(np.asarray(w_qkv), dtype=np.float32)
    w_out = np.ascontiguousarray(np.asarray(w_out), dtype=np.float32)

    nc = _get_compiled()

    xT = np.ascontiguousarray(x.reshape(C, N))
    in_maps = []
    for h in range(NCORES):
        rows = np.concatenate([
            np.arange(h * DH, (h + 1) * DH),
            np.arange(C + h * DH, C + (h + 1) * DH),
            np.arange(2 * C + h * DH, 2 * C + (h + 1) * DH),
        ])
        wqkvT = np.ascontiguousarray(w_qkv[rows, :].T)            # [512, 192]
        w_outT = np.ascontiguousarray(w_out[:, h * DH:(h + 1) * DH].T)  # [64, 512]
        in_maps.append({"xT": xT, "wqkvT": wqkvT, "w_outT": w_outT})

    res = run_bass_kernel_spmd(nc, in_maps, core_ids=list(range(NCORES)))

    out = np.zeros((C, N), dtype=np.float32)
    for r in res.results:
        out += r["outp"]
    return out.reshape(1, C, 16, 16, 16)


# revision 13
# speedup vs baseline: 1.3013x; 1.3013x over previous
"""Multi-head 3D attention (8 heads, C=512, N=16^3=4096) on 8 Trainium2 cores.

Sharding: one head per NeuronCore (head-parallel). Each core receives the
full token activations plus its head's slice of the qkv/out projection
weights, computes its head's attention and its partial contribution to the
output projection; the host sums the 8 partial outputs.

Per-core algorithm:
  xT   = x.reshape(C, N)                      # [512, 4096] fp16, channel-major
  qT   = Wq @ xT, kT = Wk @ xT                # [64, 4096] fp16 (dh on partitions)
  v    = xT.T @ Wv.T                          # [4096, 64] bf16 (keys on partitions)
  S^T  = kT.T-tiles @ qT                      # [128 keys, 1024 q] PSUM fp32
  P^T  = exp(8 * S^T)                         # ACT, scale fused, bf16, no max-sub
  o_aug= [v, 1].T @ P^T                       # [65, 2048] PSUM; row 64 = denom
  o    = o_aug[:64] * (1/denom)               # fp16 (normalized -> fp16 safe)
  outp = w_out_h @ o                          # [512, 4096] fp32 partial

Precision notes: fp16 (11-bit mantissa) for q/k keeps logit error ~1e-2 abs
(logits reach +-80, softmax is very peaked, so bf16 there is NOT ok). P is
bf16 because unnormalized exp reaches e^75 which overflows fp16's range.
2-byte matmul operands stream at 1 cycle/row with N=1024 moving (4-byte
fp32/f32r only manage 2 cycles/row, N<=512). No softmax max-subtraction:
the HW exp is accurate over the whole fp32 range and e^75 fits fp32/bf16.
Host-simulated end-to-end absmax relative error: ~3.7e-3.
"""

import sys

for _p in ("/opt/trn_rl_repo",):
    if _p not in sys.path:
        sys.path.insert(0, _p)

import numpy as np

C = 512          # channels
N = 4096         # tokens (16*16*16)
HEADS = 8
DH = C // HEADS  # 64
SCALE = float(DH) ** 0.5  # 8.0 (reference multiplies by sqrt(dh))
NCORES = 8

KT = 128                 # key-tile size (S^T partition dim)
NKT = N // KT            # 32
QG = 2048                # queries per o-psum accumulation group
NQG = N // QG            # 2
SW = 1024                # S-tile width (queries per exp call / S matmul)
NSW = QG // SW           # 2
MV = 512                 # max matmul free dim (one PSUM bank)

_compiled = None


def _build():
    import concourse.tile as tile
    from concourse import bacc, mybir

    F32 = mybir.dt.float32
    F16 = mybir.dt.float16
    BF16 = mybir.dt.bfloat16
    EXP = mybir.ActivationFunctionType.Exp

    nc = bacc.Bacc("TRN2", num_devices=NCORES)
    xT_d = nc.dram_tensor("xT", [C, N], F16, kind="ExternalInput")
    # columns 0:64 = Wq^T, 64:128 = Wk^T, 128:192 = Wv^T (this head's rows)
    wqkvT_d = nc.dram_tensor("wqkvT", [C, 3 * DH], F16, kind="ExternalInput")
    # w_out[:, head_cols].T  -> [64, 512]
    w_outT_d = nc.dram_tensor("w_outT", [DH, C], F16, kind="ExternalInput")
    outp_d = nc.dram_tensor("outp", [C, N], F32, kind="ExternalOutput")

    NCT = C // 128  # 4 channel tiles

    with tile.TileContext(nc) as tc:
        with tc.tile_pool(name="const", bufs=1) as const:
            # ---- persistent SBUF tensors ----
            xt = [const.tile([128, N], F16, tag=f"x{i}", name=f"x{i}")
                  for i in range(NCT)]
            wqkv = [const.tile([128, 3 * DH], F16, tag=f"w{i}", name=f"w{i}")
                    for i in range(NCT)]
            woutT = const.tile([DH, C], F16, tag="wo")
            qT = const.tile([DH, N], F16, tag="qT")
            kT = const.tile([DH, N], F16, tag="kT")
            vaug = const.tile([128, NKT, DH + 1], BF16, tag="vaug")
            o_sb = const.tile([DH, N], F16, tag="o")         # normalized o^T
            recip = const.tile([1, N], F32, tag="recip")     # 1/denominator
            recipb = const.tile([DH, N], F32, tag="recipb")  # broadcast to 64p
            ones = const.tile([128, 1], F32, tag="ones")
            nc.vector.memset(ones, 1.0)

            for i in range(NCT):
                nc.sync.dma_start(out=xt[i], in_=xT_d.ap()[i * 128:(i + 1) * 128, :])
                nc.sync.dma_start(out=wqkv[i], in_=wqkvT_d.ap()[i * 128:(i + 1) * 128, :])
            nc.sync.dma_start(out=woutT, in_=w_outT_d.ap())

            # ---- phase 1: projections ----
            with tc.tile_pool(name="ph1", bufs=2, space="PSUM") as ph1:
                # qT / kT: [64, 4096] = Wq/Wk @ xT, by token chunks of 512
                for ch in range(N // 512):
                    sl = slice(ch * 512, (ch + 1) * 512)
                    psq = ph1.tile([DH, 512], F32, tag="psq")
                    psk = ph1.tile([DH, 512], F32, tag="psk")
                    for ct in range(NCT):
                        nc.tensor.matmul(psq, lhsT=wqkv[ct][:, 0:DH],
                                         rhs=xt[ct][:, sl],
                                         start=(ct == 0), stop=(ct == NCT - 1))
                    for ct in range(NCT):
                        nc.tensor.matmul(psk, lhsT=wqkv[ct][:, DH:2 * DH],
                                         rhs=xt[ct][:, sl],
                                         start=(ct == 0), stop=(ct == NCT - 1))
                    nc.vector.tensor_copy(out=qT[:, sl], in_=psq)
                    nc.vector.tensor_copy(out=kT[:, sl], in_=psk)
                # v: [4096, 64] keys-on-partitions, one [128, 64] tile per key tile
                for kt_i in range(NKT):
                    psv = ph1.tile([128, DH], F32, tag="psv")
                    for ct in range(NCT):
                        nc.tensor.matmul(psv,
                                         lhsT=xt[ct][:, kt_i * KT:(kt_i + 1) * KT],
                                         rhs=wqkv[ct][:, 2 * DH:3 * DH],
                                         start=(ct == 0), stop=(ct == NCT - 1))
                    nc.vector.tensor_copy(out=vaug[:, kt_i, 0:DH], in_=psv)
                    nc.vector.tensor_copy(out=vaug[:, kt_i, DH:DH + 1], in_=ones)

            # ---- phase 2: attention ----
            with tc.tile_pool(name="s_ps", bufs=2, space="PSUM") as s_ps, \
                 tc.tile_pool(name="o_ps", bufs=1, space="PSUM") as o_ps, \
                 tc.tile_pool(name="p_sb", bufs=3) as p_sb:
                for qg in range(NQG):
                    q0 = qg * QG
                    ops = o_ps.tile([DH + 1, QG], F32, tag="ops")
                    for kt_i in range(NKT):
                        ksl = slice(kt_i * KT, (kt_i + 1) * KT)
                        for sw in range(NSW):
                            s0 = sw * SW
                            sps = s_ps.tile([128, SW], F32, tag="s")
                            for mv in range(SW // MV):
                                nc.tensor.matmul(
                                    sps[:, mv * MV:(mv + 1) * MV],
                                    lhsT=kT[:, ksl],
                                    rhs=qT[:, q0 + s0 + mv * MV:
                                           q0 + s0 + (mv + 1) * MV],
                                    start=True, stop=True)
                            pt = p_sb.tile([128, SW], BF16, tag="p")
                            nc.scalar.activation(out=pt, in_=sps, func=EXP,
                                                 scale=SCALE)
                            for mv in range(SW // MV):
                                nc.tensor.matmul(
                                    ops[:, s0 + mv * MV: s0 + (mv + 1) * MV],
                                    lhsT=vaug[:, kt_i, :],
                                    rhs=pt[:, mv * MV:(mv + 1) * MV],
                                    start=(kt_i == 0),
                                    stop=(kt_i == NKT - 1))
                    # normalize: o = o_aug[:64] / denom  (fp16, fused into flush)
                    gsl = slice(q0, q0 + QG)
                    nc.vector.reciprocal(out=recip[:, gsl],
                                         in_=ops[DH:DH + 1, :])
                    nc.gpsimd.partition_broadcast(recipb[:, gsl], recip[:, gsl])
                    nc.vector.tensor_mul(o_sb[:, gsl], ops[0:DH, :], recipb[:, gsl])

            # ---- phase 3: output projection ----
            with tc.tile_pool(name="out_ps", bufs=2, space="PSUM") as out_ps, \
                 tc.tile_pool(name="out_sb", bufs=3) as out_sb:
                for ch in range(N // MV):
                    for ct in range(NCT):
                        sl = slice(ch * MV, (ch + 1) * MV)
                        pso = out_ps.tile([128, MV], F32, tag="pso")
                        nc.tensor.matmul(pso,
                                         lhsT=woutT[:, ct * 128:(ct + 1) * 128],
                                         rhs=o_sb[:, sl], start=True, stop=True)
                        ot = out_sb.tile([128, MV], F32, tag="ot")
                        nc.vector.tensor_copy(out=ot, in_=pso)
                        nc.sync.dma_start(
                            out=outp_d.ap()[ct * 128:(ct + 1) * 128, sl], in_=ot)

    nc.compile()
    return nc


def _get_compiled():
    global _compiled
    if _compiled is None:
        _compiled = _build()
    return _compiled


def make_in_maps(x, w_qkv, w_out):
    xT = np.ascontiguousarray(x.reshape(C, N).astype(np.float16))
    in_maps = []
    for h in range(NCORES):
        rows = np.concatenate([
            np.arange(h * DH, (h + 1) * DH),
            np.arange(C + h * DH, C + (h + 1) * DH),
            np.arange(2 * C + h * DH, 2 * C + (h + 1) * DH),
        ])
        wqkvT = np.ascontiguousarray(w_qkv[rows, :].T.astype(np.float16))
        w_outT = np.ascontiguousarray(
            w_out[:, h * DH:(h + 1) * DH].T.astype(np.float16))
        in_maps.append({"xT": xT, "wqkvT": wqkvT, "w_outT": w_outT})
    return in_maps


def kernel(x, w_qkv, w_out):
    from concourse.bass_utils import run_bass_kernel_spmd

    x = np.ascontiguousarray(np.asarray(x), dtype=np.float32)
    w_qkv = np.ascontiguousarray(np.asarray(w_qkv), dtype=np.float32)
    w_out = np.ascontiguousarray(np.asarray(w_out), dtype=np.float32)

    nc = _get_compiled()
    res = run_bass_kernel_spmd(nc, make_in_maps(x, w_qkv, w_out),
                               core_ids=list(range(NCORES)))

    out = np.zeros((C, N), dtype=np.float32)
    for r in res.results:
        out += r["outp"]
    return out.reshape(1, C, 16, 16, 16)


# revision 16
# speedup vs baseline: 1.4463x; 1.1114x over previous
"""Multi-head 3D attention (8 heads, C=512, N=16^3=4096) on 8 Trainium2 cores.

Sharding: one head per NeuronCore (head-parallel). Each core receives the
full token activations plus its head's slice of the qkv/out projection
weights, computes its head's attention and its partial contribution to the
output projection; the host sums the 8 partial outputs.

Per-core algorithm:
  xT   = x.reshape(C, N)                      # [512, 4096] fp16, channel-major
  qT   = Wq @ xT, kT = Wk @ xT                # [64, 4096] fp16 (dh on partitions)
  v    = xT.T @ Wv.T                          # [4096, 64] bf16 (keys on partitions)
  S^T  = kT.T-tiles @ qT                      # [128 keys, 1024 q] PSUM fp32
  P^T  = exp(8 * S^T)                         # ACT, scale fused, bf16, no max-sub
  o_aug= [v, 1].T @ P^T                       # [65, 2048] PSUM; row 64 = denom
  o    = o_aug[:64] * (1/denom)               # fp16 (normalized -> fp16 safe)
  outp = w_out_h @ o                          # [512, 4096] fp32 partial

Precision notes: fp16 (11-bit mantissa) for q/k keeps logit error ~1e-2 abs
(logits reach +-80, softmax is very peaked, so bf16 there is NOT ok). P is
bf16 because unnormalized exp reaches e^75 which overflows fp16's range.
2-byte matmul operands stream at 1 cycle/row with N=1024 moving (4-byte
fp32/f32r only manage 2 cycles/row, N<=512). No softmax max-subtraction:
the HW exp is accurate over the whole fp32 range and e^75 fits fp32/bf16.
Host-simulated end-to-end absmax relative error: ~3.7e-3.
"""

import sys

for _p in ("/opt/trn_rl_repo",):
    if _p not in sys.path:
        sys.path.insert(0, _p)

import numpy as np

C = 512          # channels
N = 4096         # tokens (16*16*16)
HEADS = 8
DH = C // HEADS  # 64
SCALE = float(DH) ** 0.5  # 8.0 (reference multiplies by sqrt(dh))
NCORES = 8

KT = 128                 # key-tile size (S^T partition dim)
NKT = N // KT            # 32
QG = 1024                # queries per o-psum accumulation group
NQG = N // QG            # 4
SW = 1024                # S-tile width (queries per exp call)
NSW = QG // SW           # 1
MV = 512                 # max matmul free dim (one PSUM bank)

_compiled = None


def _build():
    import concourse.tile as tile
    from concourse import bacc, mybir

    F32 = mybir.dt.float32
    F16 = mybir.dt.float16
    BF16 = mybir.dt.bfloat16
    EXP = mybir.ActivationFunctionType.Exp

    nc = bacc.Bacc("TRN2", num_devices=NCORES)
    xT_d = nc.dram_tensor("xT", [C, N], F16, kind="ExternalInput")
    # columns 0:64 = Wq^T, 64:128 = Wk^T, 128:192 = Wv^T (this head's rows)
    wqkvT_d = nc.dram_tensor("wqkvT", [C, 3 * DH], F16, kind="ExternalInput")
    # w_out[:, head_cols].T  -> [64, 512]
    w_outT_d = nc.dram_tensor("w_outT", [DH, C], F16, kind="ExternalInput")
    outp_d = nc.dram_tensor("outp", [C, N], F32, kind="ExternalOutput")

    NCT = C // 128  # 4 channel tiles

    with tile.TileContext(nc) as tc:
        with tc.tile_pool(name="const", bufs=1) as const:
            # ---- persistent SBUF tensors ----
            xt = [const.tile([128, N], F16, tag=f"x{i}", name=f"x{i}")
                  for i in range(NCT)]
            wqkv = [const.tile([128, 3 * DH], F16, tag=f"w{i}", name=f"w{i}")
                    for i in range(NCT)]
            woutT = const.tile([DH, C], F16, tag="wo")
            qT = const.tile([DH, N], F16, tag="qT")
            kT = const.tile([DH, N], F16, tag="kT")
            vaug = const.tile([128, NKT, DH + 1], BF16, tag="vaug")
            o_sb = const.tile([DH, N], F16, tag="o")         # normalized o^T
            recip = const.tile([1, N], F32, tag="recip")     # 1/denominator
            recipb = const.tile([DH, N], F32, tag="recipb")  # broadcast to 64p
            ones = const.tile([128, 1], F32, tag="ones")
            nc.vector.memset(ones, 1.0)

            for i in range(NCT):
                nc.sync.dma_start(out=xt[i], in_=xT_d.ap()[i * 128:(i + 1) * 128, :])
                nc.sync.dma_start(out=wqkv[i], in_=wqkvT_d.ap()[i * 128:(i + 1) * 128, :])
            nc.sync.dma_start(out=woutT, in_=w_outT_d.ap())

            # ---- phase 1: projections ----
            with tc.tile_pool(name="ph1", bufs=2, space="PSUM") as ph1:
                # qT / kT: [64, 4096] = Wq/Wk @ xT, by token chunks of 512
                for ch in range(N // 512):
                    sl = slice(ch * 512, (ch + 1) * 512)
                    psq = ph1.tile([DH, 512], F32, tag="psq")
                    psk = ph1.tile([DH, 512], F32, tag="psk")
                    for ct in range(NCT):
                        nc.tensor.matmul(psq, lhsT=wqkv[ct][:, 0:DH],
                                         rhs=xt[ct][:, sl],
                                         start=(ct == 0), stop=(ct == NCT - 1))
                    for ct in range(NCT):
                        nc.tensor.matmul(psk, lhsT=wqkv[ct][:, DH:2 * DH],
                                         rhs=xt[ct][:, sl],
                                         start=(ct == 0), stop=(ct == NCT - 1))
                    nc.vector.tensor_copy(out=qT[:, sl], in_=psq)
                    nc.vector.tensor_copy(out=kT[:, sl], in_=psk)
                # v: [4096, 64] keys-on-partitions, one [128, 64] tile per key tile
                for kt_i in range(NKT):
                    psv = ph1.tile([128, DH], F32, tag="psv")
                    for ct in range(NCT):
                        nc.tensor.matmul(psv,
                                         lhsT=xt[ct][:, kt_i * KT:(kt_i + 1) * KT],
                                         rhs=wqkv[ct][:, 2 * DH:3 * DH],
                                         start=(ct == 0), stop=(ct == NCT - 1))
                    nc.scalar.copy(out=vaug[:, kt_i, 0:DH], in_=psv)
                    nc.scalar.copy(out=vaug[:, kt_i, DH:DH + 1], in_=ones)

            # ---- phase 2: attention ----
            # Software-pipelined emission: the P @ v matmuls for key tile
            # kt are emitted one iteration behind the S matmuls/exp for
            # kt+1, so the PE never sits at a wait for the exp it just
            # triggered and streams matmuls densely (keeps HAM warm).
            with tc.tile_pool(name="s_ps", bufs=3, space="PSUM") as s_ps, \
                 tc.tile_pool(name="o_ps", bufs=1, space="PSUM") as o_ps, \
                 tc.tile_pool(name="p_sb", bufs=6) as p_sb:
                for qg in range(NQG):
                    q0 = qg * QG
                    ops = o_ps.tile([DH + 1, QG], F32, tag="ops",
                                    name=f"ops{qg}")
                    pts = {}
                    for kt_i in range(NKT + 1):
                        if kt_i < NKT:
                            ksl = slice(kt_i * KT, (kt_i + 1) * KT)
                            sps = s_ps.tile([128, SW], F32, tag="s",
                                            name=f"sps{qg}_{kt_i}")
                            for mv in range(SW // MV):
                                nc.tensor.matmul(
                                    sps[:, mv * MV:(mv + 1) * MV],
                                    lhsT=kT[:, ksl],
                                    rhs=qT[:, q0 + mv * MV: q0 + (mv + 1) * MV],
                                    start=True, stop=True)
                            pt = p_sb.tile([128, SW], BF16, tag="p",
                                           name=f"pt{qg}_{kt_i}")
                            nc.scalar.activation(out=pt, in_=sps, func=EXP,
                                                 scale=SCALE)
                            pts[kt_i] = pt
                        if kt_i >= 1:
                            ot_i = kt_i - 1
                            pt = pts.pop(ot_i)
                            for mv in range(SW // MV):
                                nc.tensor.matmul(
                                    ops[:, mv * MV:(mv + 1) * MV],
                                    lhsT=vaug[:, ot_i, :],
                                    rhs=pt[:, mv * MV:(mv + 1) * MV],
                                    start=(ot_i == 0),
                                    stop=(ot_i == NKT - 1))
                    # normalize: o = o_aug[:64] / denom  (fp16, fused into flush)
                    gsl = slice(q0, q0 + QG)
                    nc.vector.reciprocal(out=recip[:, gsl],
                                         in_=ops[DH:DH + 1, :])
                    nc.gpsimd.partition_broadcast(recipb[:, gsl], recip[:, gsl])
                    nc.vector.tensor_mul(o_sb[:, gsl], ops[0:DH, :], recipb[:, gsl])

            # ---- phase 3: output projection ----
            with tc.tile_pool(name="out_ps", bufs=2, space="PSUM") as out_ps, \
                 tc.tile_pool(name="out_sb", bufs=3) as out_sb:
                for ch in range(N // MV):
                    for ct in range(NCT):
                        sl = slice(ch * MV, (ch + 1) * MV)
                        pso = out_ps.tile([128, MV], F32, tag="pso")
                        nc.tensor.matmul(pso,
                                         lhsT=woutT[:, ct * 128:(ct + 1) * 128],
                                         rhs=o_sb[:, sl], start=True, stop=True)
                        ot = out_sb.tile([128, MV], F32, tag="ot")
                        nc.vector.tensor_copy(out=ot, in_=pso)
                        nc.sync.dma_start(
                            out=outp_d.ap()[ct * 128:(ct + 1) * 128, sl], in_=ot)

    nc.compile()
    return nc


def _get_compiled():
    global _compiled
    if _compiled is None:
        _compiled = _build()
    return _compiled


def make_in_maps(x, w_qkv, w_out):
    xT = np.ascontiguousarray(x.reshape(C, N).astype(np.float16))
    in_maps = []
    for h in range(NCORES):
        rows = np.concatenate([
            np.arange(h * DH, (h + 1) * DH),
            np.arange(C + h * DH, C + (h + 1) * DH),
            np.arange(2 * C + h * DH, 2 * C + (h + 1) * DH),
        ])
        wqkvT = np.ascontiguousarray(w_qkv[rows, :].T.astype(np.float16))
        w_outT = np.ascontiguousarray(
            w_out[:, h * DH:(h + 1) * DH].T.astype(np.float16))
        in_maps.append({"xT": xT, "wqkvT": wqkvT, "w_outT": w_outT})
    return in_maps


def kernel(x, w_qkv, w_out):
    from concourse.bass_utils import run_bass_kernel_spmd

    x = np.ascontiguousarray(np.asarray(x), dtype=np.float32)
    w_qkv = np.ascontiguousarray(np.asarray(w_qkv), dtype=np.float32)
    w_out = np.ascontiguousarray(np.asarray(w_out), dtype=np.float32)

    nc = _get_compiled()
    res = run_bass_kernel_spmd(nc, make_in_maps(x, w_qkv, w_out),
                               core_ids=list(range(NCORES)))

    out = np.zeros((C, N), dtype=np.float32)
    for r in res.results:
        out += r["outp"]
    return out.reshape(1, C, 16, 16, 16)
